# revision 36
# baseline (speedup 1.0000x reference)
"""CRF tagger loss kernel for Trainium2 (8 NeuronCores, data-parallel over batch).

Self-contained: hardcodes all shapes. kernel(**inputs) takes full inputs,
shards batch over 8 cores, runs one SPMD Bass program, returns [B] f32 loss.

Embedding gather: two-stage dma_gather with descriptor generation spread
across all 4 SWDGE queues (each queue runs on its own Q7 core pair, so the
4 queues generate descriptors concurrently). Stage 1 gathers bf16 table
rows chunk-compactly (int16 row index per 32768-row chunk, zero-padded to
static counts); stage 2 un-permutes via SBUF-source transpose gathers
(split over the queues so that per-queue descriptor counts balance:
2304+1024 = 256+3072 = 3328 per queue per group).

Forward algorithm: split at the sequence midpoint into a forward chain
(alpha, 256 steps) and a backward chain (beta, 256 steps) that run
concurrently, halving the serial latency. Both run in the scaled domain
(bf16 state, f32 PSUM) with a log accumulator and periodic rescale:
  fwd: A_t = (expT^T A_{t-1}) * E_t
  bwd: Z_{t-1} = expT (E_t * Z_t)
  denom = ln(sum_i A_mid[i] * Z_mid[i]) + Lf + Lb + S*lnT
"""
import os
import sys

sys.path.insert(0, "/opt/trn_rl_repo")

import numpy as np
import ml_dtypes

import concourse.bacc as bacc
import concourse.bass as bass
import concourse.tile as tile
from concourse import mybir
from concourse.bass import AP

# ---- problem dims (hardcoded from the nn_CRFTagger problem) ----
B, S, W, V, E, H, T = 512, 512, 3, 100000, 128, 100, 64
NCORES = 8
BC = B // NCORES          # sequences per core = 64
N = BC * S                # tokens per core = 32768 (time-major: t = s*BC + b)
GTOK = 2048               # tokens per gather group
NGG = N // GTOK           # gather groups = 16
LK = 3 * GTOK             # lookups per group = 6144
CHUNK = 32768             # table rows addressable per int16 gather
TC = [2176, 2176, 2176, 256]   # static per-chunk gather capacity
TSTART = [0, 2176, 4352, 6528]
NSTAGE = 6784             # staged slots per group (= sum(TC), 53*128)
S2SPLIT = [1024, 1024, 1024, 3072]   # stage-2 idx per queue (128-aligned)
IXW = sum(c // 16 for c in TC) + 3 * 2048 // 16   # idx cols per group = 808
WIN = 512                 # tokens per window (= 8 time steps x 64 b)
NW = N // WIN             # windows = 64
NWH = NW // 2             # windows per scan direction = 32
WPG = GTOK // WIN         # windows per group = 4
SPW = WIN // BC           # time steps per window = 8
LA = 1                    # window lookahead (emission ahead of scan)
EMBUFS = 8                # rotating emission-window buffers
RESCALE = 32              # scan rescale cadence
F32 = mybir.dt.float32
BF16 = mybir.dt.bfloat16
PAIR_PAD = T * T          # dummy pair index -> gathers 0.0
NPAIR = (S * BC) // NCORES   # pairs per 16-partition stripe = 4096
PCHUNK = 4                # pair-gather chunks
NQ = 4                    # SWDGE queues


def build_program():
    nc = bacc.Bacc("TRN2", target_bir_lowering=False, debug=False,
                   num_swdge_queues=NQ)

    # ---- DRAM I/O ----
    idx_d = nc.dram_tensor("idx", [128, NGG * IXW], mybir.dt.int16,
                           kind="ExternalInput")
    ohm_d = nc.dram_tensor("ohmb", [T, N], BF16, kind="ExternalInput")
    pairs_d = nc.dram_tensor("pairs", [128, NPAIR // 16],
                             mybir.dt.int16, kind="ExternalInput")
    table_d = nc.dram_tensor("tableb", [V, E], BF16, kind="ExternalInput")
    params_d = nc.dram_tensor("params", [128, 5], F32, kind="ExternalInput")
    w1b_d = nc.dram_tensor("w1b", [E, H], BF16, kind="ExternalInput")
    w2b_d = nc.dram_tensor("w2b", [H, T], BF16, kind="ExternalInput")
    trans_d = nc.dram_tensor("trans", [T, T], F32, kind="ExternalInput")
    transT_d = nc.dram_tensor("transT", [T, T], F32, kind="ExternalInput")
    tflat_d = nc.dram_tensor("tflat", [128, T * T + 1], F32,
                             kind="ExternalInput")
    bd_d = nc.dram_tensor("bd", [128, 8], F32, kind="ExternalInput")
    rows_d = nc.dram_tensor("rows", [1, 2 * T], F32, kind="ExternalInput")
    out_d = nc.dram_tensor("out", [1, BC], F32, kind="ExternalOutput")

    with tile.TileContext(nc) as tc:
        with (
            tc.tile_pool(name="const", bufs=1) as cp,
            tc.tile_pool(name="emtiles", bufs=6) as emp,
            tc.tile_pool(name="stage", bufs=6) as stp,
            tc.tile_pool(name="ixp", bufs=6) as ixp,
            tc.tile_pool(name="g2p", bufs=4) as g2p,
            tc.tile_pool(name="hpool", bufs=3) as hp,
            tc.tile_pool(name="ohm", bufs=4) as op_,
            tc.tile_pool(name="upool", bufs=8) as up,
            tc.tile_pool(name="small", bufs=4) as sp,
            tc.tile_pool(name="pgp", bufs=4) as pgp,
            tc.tile_pool(name="psW", bufs=4, space="PSUM") as psW,
            tc.tile_pool(name="psS", bufs=4, space="PSUM") as psS,
        ):
            # ---- constants to SBUF ----
            params = cp.tile([128, 5], F32)
            nc.sync.dma_start(out=params[:], in_=params_d[:])
            w1b = cp.tile([E, H], BF16)
            nc.sync.dma_start(out=w1b[:], in_=w1b_d[:])
            w2b = cp.tile([H, T], BF16)
            nc.sync.dma_start(out=w2b[:], in_=w2b_d[:])
            trans = cp.tile([T, T], F32)
            nc.sync.dma_start(out=trans[:], in_=trans_d[:])
            transT = cp.tile([T, T], F32)
            nc.sync.dma_start(out=transT[:], in_=transT_d[:])
            bd = cp.tile([128, 8], F32)
            nc.sync.dma_start(out=bd[:], in_=bd_d[:])
            pairs = cp.tile([128, NPAIR // 16], mybir.dt.int16)
            nc.sync.dma_start(out=pairs[:], in_=pairs_d[:])
            tflat = cp.tile([128, T * T + 1], F32)
            rows = cp.tile([1, 2 * T], F32)
            nc.sync.dma_start(out=rows[:], in_=rows_d[:])

            expT = cp.tile([T, T], BF16)
            nc.scalar.activation(out=expT[:], in_=trans[:],
                                 func=mybir.ActivationFunctionType.Exp)
            expTT = cp.tile([T, T], BF16)
            nc.scalar.activation(out=expTT[:], in_=transT[:],
                                 func=mybir.ActivationFunctionType.Exp)
            # exp(start/end) broadcast to [T, BC] via outer-product matmul
            # (tensor_scalar with a per-partition scalar vector is ~50us on
            # DVE hardware -- avoid it on the critical path).
            expRows = cp.tile([1, 2 * T], F32)
            nc.scalar.activation(out=expRows[:], in_=rows[:],
                                 func=mybir.ActivationFunctionType.Exp)
            onesb_row = cp.tile([1, BC], F32)
            nc.vector.memset(onesb_row[:], 1.0)
            expStartB = cp.tile([T, BC], BF16)
            expEndB = cp.tile([T, BC], BF16)
            for _i, _dst in ((0, expStartB), (1, expEndB)):
                _ps = psS.tile([T, BC], F32, tag="scan")
                nc.tensor.matmul(_ps[:], lhsT=expRows[0:1, _i * T:
                                                       (_i + 1) * T],
                                 rhs=onesb_row[:])
                nc.vector.tensor_copy(out=_dst[:], in_=_ps[:])
            stb = cp.tile([T, 1], BF16)          # start_trans bf16
            nc.vector.tensor_copy(out=stb[:], in_=params[0:T, 0:1])
            etb = cp.tile([T, 1], BF16)          # end_trans bf16
            nc.vector.tensor_copy(out=etb[:], in_=params[0:T, 1:2])

            ones_rb = cp.tile([1, T], BF16)
            nc.vector.memset(ones_rb[:], 1.0)
            ones_rf = cp.tile([1, T], F32)
            nc.vector.memset(ones_rf[:], 1.0)
            ones_cb = cp.tile([T, 1], BF16)
            nc.vector.memset(ones_cb[:], 1.0)
            ones_cf = cp.tile([T, 1], F32)
            nc.vector.memset(ones_cf[:], 1.0)

            acc2 = cp.tile([T, BC], F32)       # sum_s ohm*(em+b2) accumulator
            nc.vector.memset(acc2[:], 0.0)
            # rescale row-0 values, logged in one batched Ln at the end
            # (fwd events in cols [0:8*BC), bwd in [8*BC:16*BC))
            lnbuf = cp.tile([1, 16 * BC], F32)
            nc.vector.memset(lnbuf[:], 1.0)
            se0 = cp.tile([1, BC], F32)        # start-term
            se1 = cp.tile([1, BC], F32)        # end-term
            nc.vector.memset(se0[:], 0.0)
            nc.vector.memset(se1[:], 0.0)
            tsum = cp.tile([1, BC], F32)
            nc.vector.memset(tsum[:], 0.0)

            st_tiles = {}
            ix_tiles = {}
            g2_tiles = {}
            em_w = {}
            fwd = {"A": None}
            bwd = {"y": None}

            def stage1(g):
                ix = ixp.tile([128, IXW], mybir.dt.int16, tag="ix")
                nc.sync.dma_start(out=ix[:], in_=idx_d[:, g * IXW:
                                                       (g + 1) * IXW])
                ix_tiles[g] = ix
                st = stp.tile([128, NSTAGE], BF16, tag="st")
                stv = st[:].rearrange("p (r e) -> p r e", e=128)
                # queue order must match stage2's (0,1,2,3): the 8 DMASW
                # sems rotate round-robin and each locks to one SWDGE queue.
                for c in (0, 1, 2, 3):
                    xoff = sum(TC[cc] // 16 for cc in range(c))
                    rows = min(CHUNK, V - c * CHUNK)
                    nc.gpsimd.dma_gather(
                        out_ap=stv[:, TSTART[c] // 128:
                                   (TSTART[c] + TC[c]) // 128, :],
                        in_ap=table_d[c * CHUNK:c * CHUNK + rows, :],
                        idxs_ap=ix[:, xoff:xoff + TC[c] // 16],
                        num_idxs=TC[c], num_idxs_reg=TC[c], elem_size=E,
                        single_packet=False, queue_num=c % NQ)
                st_tiles[g] = st

            def stage2(g):
                st = st_tiles.pop(g)
                ix = ix_tiles.pop(g)
                s2base = sum(c // 16 for c in TC)
                g2 = g2p.tile([128, 1, LK], BF16, tag="g2")
                off = 0
                for q in range(NQ):
                    n = S2SPLIT[q]
                    xoff = s2base + off // 16
                    nc.gpsimd.dma_gather(
                        out_ap=g2[:, :, off:off + n],
                        in_ap=st[:], idxs_ap=ix[:, xoff:xoff + n // 16],
                        num_idxs=n, num_idxs_reg=n, elem_size=E,
                        transpose=True, single_packet=False, queue_num=q,
                        sbuf_tokens_per_rank=128, sbuf_free_dim_per_rank=256,
                        sbuf_free_dim_pad_per_rank=0, sbuf_byte_offset=0)
                    off += n
                g2_tiles[g] = g2

            def emission(w):
                g2 = g2_tiles[w // WPG]
                col = (w % WPG) * WIN
                h_ps = psW.tile([H, WIN], F32, tag="psw")
                for i in range(W):
                    nc.tensor.matmul(h_ps[:], lhsT=w1b[:],
                                     rhs=g2[:, 0, i * GTOK + col:
                                            i * GTOK + col + WIN],
                                     start=(i == 0), stop=(i == W - 1))
                h_sb = hp.tile([H, WIN], BF16, tag="h")
                nc.scalar.activation(out=h_sb[:], in_=h_ps[:],
                                     func=mybir.ActivationFunctionType.Tanh,
                                     bias=params[0:H, 2:3])
                em_ps = psW.tile([T, WIN], F32, tag="psw")
                nc.tensor.matmul(em_ps[:], lhsT=w2b[:], rhs=h_sb[:])
                # exp(em + b2 - logT) -> emission-exp window (rotating buffer)
                emt = emp.tile([T, WIN], BF16, tag="em")
                em_w[w] = emt
                nc.scalar.activation(out=emt[:], in_=em_ps[:],
                                     func=mybir.ActivationFunctionType.Exp,
                                     bias=params[0:T, 3:4])
                # numerator: gold-path emission scores via host one-hot tags
                ohm = op_.tile([T, WIN], BF16, tag="ohm")
                nc.sync.dma_start(out=ohm[:],
                                  in_=ohm_d[:, w * WIN:(w + 1) * WIN])
                tmp = op_.tile([T, WIN], BF16, tag="tmp")
                nc.vector.scalar_tensor_tensor(
                    out=tmp[:], in0=em_ps[:], scalar=params[0:T, 4:5],
                    in1=ohm[:], op0=mybir.AluOpType.add,
                    op1=mybir.AluOpType.mult)
                red = op_.tile([T, BC], F32, tag="red")
                nc.vector.tensor_reduce(
                    out=red[:],
                    in_=tmp[:].rearrange("p (s b) -> p b s", s=SPW),
                    axis=mybir.AxisListType.X, op=mybir.AluOpType.add)
                nc.vector.tensor_tensor(out=acc2[:], in0=acc2[:], in1=red[:],
                                        op=mybir.AluOpType.add)
                if w == 0:
                    s0_ps = psS.tile([1, BC], F32, tag="scan")
                    nc.tensor.matmul(s0_ps[:], lhsT=stb[:],
                                     rhs=ohm[:, 0:BC])
                    nc.vector.tensor_copy(out=se0[:], in_=s0_ps[:])
                if w == NW - 1:
                    s1_ps = psS.tile([1, BC], F32, tag="scan")
                    nc.tensor.matmul(s1_ps[:], lhsT=etb[:],
                                     rhs=ohm[:, WIN - BC:WIN])
                    nc.vector.tensor_copy(out=se1[:], in_=s1_ps[:])

            def rescale(state, slot):
                """Divide state (SBUF [T,BC] bf16) by its row 0; stash the
                row-0 values in lnbuf slot for one batched Ln at the end."""
                rec = sp.tile([1, BC], F32, tag="rec")
                nc.vector.reciprocal(out=rec[:], in_=state[0:1, :])
                nc.vector.tensor_copy(
                    out=lnbuf[0:1, slot * BC:(slot + 1) * BC],
                    in_=state[0:1, :])
                rb_ps = psS.tile([T, BC], F32, tag="scan")
                nc.tensor.matmul(rb_ps[:], lhsT=ones_rf[:], rhs=rec[:])
                s2 = up.tile([T, BC], BF16, tag="U")
                nc.vector.tensor_tensor(out=s2[:], in0=rb_ps[:], in1=state[:],
                                        op=mybir.AluOpType.mult)
                return s2

            def scan_fwd_window(w):
                emw = em_w.pop(w)
                for sl in range(SPW):
                    s = w * SPW + sl
                    col = sl * BC
                    if s == 0:
                        A = up.tile([T, BC], BF16, tag="U")
                        nc.vector.tensor_tensor(
                            out=A[:], in0=emw[:, col:col + BC],
                            in1=expStartB[:], op=mybir.AluOpType.mult)
                        fwd["A"] = A
                        continue
                    y_ps = psS.tile([T, BC], F32, tag="scan")
                    nc.tensor.matmul(y_ps[:], lhsT=expT[:], rhs=fwd["A"][:])
                    A = up.tile([T, BC], BF16, tag="U")
                    nc.vector.tensor_tensor(out=A[:], in0=y_ps[:],
                                            in1=emw[:, col:col + BC],
                                            op=mybir.AluOpType.mult)
                    fwd["A"] = A
                    if s % RESCALE == RESCALE - 1:
                        fwd["A"] = rescale(fwd["A"], s // RESCALE)

            def scan_bwd_window(w):
                emw = em_w.pop(w)
                for sl in reversed(range(SPW)):
                    s = w * SPW + sl
                    k = (S - 1) - s
                    col = sl * BC
                    Wt = up.tile([T, BC], BF16, tag="U")
                    if s == S - 1:
                        nc.vector.tensor_tensor(
                            out=Wt[:], in0=emw[:, col:col + BC],
                            in1=expEndB[:], op=mybir.AluOpType.mult)
                    else:
                        nc.vector.tensor_tensor(out=Wt[:], in0=bwd["y"][:],
                                                in1=emw[:, col:col + BC],
                                                op=mybir.AluOpType.mult)
                    if k % RESCALE == RESCALE - 1:
                        Wt = rescale(Wt, 8 + k // RESCALE)
                    y_ps = psS.tile([T, BC], F32, tag="scan")
                    nc.tensor.matmul(y_ps[:], lhsT=expTT[:], rhs=Wt[:])
                    bwd["y"] = y_ps

            def pairs_block():
                # numerator: transition-pair scores via ap_gather. Issued
                # right after the last dma_gather so the ap_gather ext-isa
                # IRAM swap lands in the gather tail, off the critical path.
                pred = cp.tile([128, 8], F32)
                cn = NPAIR // PCHUNK              # 512 idxs per chunk
                for c in range(PCHUNK):
                    pg = pgp.tile([128, cn], F32, tag="pg")
                    nc.gpsimd.ap_gather(
                        out_ap=pg[:].rearrange("p (n o) -> p n o", o=1),
                        in_ap=tflat[:].rearrange("p (n o) -> p n o", o=1),
                        idxs_ap=pairs[:, c * (cn // 16):(c + 1) * (cn // 16)],
                        channels=128, num_elems=T * T + 1, d=1, num_idxs=cn,
                    )
                    nc.vector.tensor_reduce(
                        out=pred[:, 2 * c:2 * c + 2],
                        in_=pg[:].rearrange("p (g s) -> p g s", g=2),
                        axis=mybir.AxisListType.X, op=mybir.AluOpType.add)
                ts_ps = psS.tile([8, 8], F32, tag="scan")
                nc.tensor.matmul(ts_ps[:], lhsT=bd[:], rhs=pred[:])
                ts8 = sp.tile([8, 8], F32)
                nc.vector.tensor_copy(out=ts8[:], in_=ts_ps[:])
                nc.sync.dma_start(
                    out=tsum[:].rearrange("p (g b) -> p g b", g=8), in_=ts8[:])

            # ---- main pipeline: emissions + fwd/bwd scans ----
            # stage1 runs two quads ahead of its stage2, so stage2's wait on
            # stage1 DMA completion is satisfied long before dispatch.
            stage1(0)
            stage1(NGG - 1)
            stage1(1)
            stage1(NGG - 2)
            for p in range(NWH + LA):
                if p < NWH and p % WPG == 0:
                    qd = p // WPG
                    stage2(qd)
                    stage2(NGG - 1 - qd)
                    if qd + 2 < NGG // 2:
                        stage1(qd + 2)
                        stage1(NGG - 3 - qd)
                    if qd == NGG // 2 - 2:
                        nc.sync.dma_start(out=tflat[:], in_=tflat_d[:])
                    if qd == NGG // 2 - 1:
                        pairs_block()
                if p < NWH:
                    emission(p)
                    emission(NW - 1 - p)
                if p >= LA:
                    scan_fwd_window(p - LA)
                    scan_bwd_window(NW - 1 - (p - LA))

            # ---- finals ----
            # denominator: ln(sum_i A_mid * Z_mid) + Lf + Lb + S*lnT
            P = sp.tile([T, BC], BF16)
            nc.vector.tensor_tensor(out=P[:], in0=bwd["y"][:],
                                    in1=fwd["A"][:], op=mybir.AluOpType.mult)
            dn_ps = psS.tile([1, BC], F32, tag="scan")
            nc.tensor.matmul(dn_ps[:], lhsT=ones_cb[:], rhs=P[:])
            dlog = sp.tile([1, BC], F32, tag="dlog")
            nc.scalar.activation(out=dlog[:], in_=dn_ps[:],
                                 func=mybir.ActivationFunctionType.Ln)
            lnv = cp.tile([1, 16 * BC], F32)
            nc.scalar.activation(out=lnv[:], in_=lnbuf[:],
                                 func=mybir.ActivationFunctionType.Ln)
            Lsum = sp.tile([1, BC], F32, tag="denom")
            nc.vector.tensor_reduce(
                out=Lsum[:],
                in_=lnv[:].rearrange("p (k b) -> p b k", k=16),
                axis=mybir.AxisListType.X, op=mybir.AluOpType.add)
            denom = sp.tile([1, BC], F32, tag="denom")
            nc.vector.tensor_tensor(out=denom[:], in0=dlog[:], in1=Lsum[:],
                                    op=mybir.AluOpType.add)
            nc.vector.tensor_scalar_add(out=denom[:], in0=denom[:],
                                        scalar1=float(S * np.log(T)))

            esc_ps = psS.tile([1, BC], F32, tag="scan")
            nc.tensor.matmul(esc_ps[:], lhsT=ones_cf[:], rhs=acc2[:])
            num = sp.tile([1, BC], F32, tag="num")
            nc.vector.tensor_tensor(out=num[:], in0=esc_ps[:], in1=tsum[:],
                                    op=mybir.AluOpType.add)
            nc.vector.tensor_tensor(out=num[:], in0=num[:], in1=se0[:],
                                    op=mybir.AluOpType.add)
            nc.vector.tensor_tensor(out=num[:], in0=num[:], in1=se1[:],
                                    op=mybir.AluOpType.add)
            outv = sp.tile([1, BC], F32, tag="outv")
            nc.vector.tensor_tensor(out=outv[:], in0=denom[:], in1=num[:],
                                    op=mybir.AluOpType.subtract)
            nc.sync.dma_start(out=out_d[:], in_=outv[:])

    nc.compile()
    return nc


def _wrap16(idx, width):
    """idx list -> [128, width] int16: i -> (partition i%16, free i//16),
    replicated across the 8 GPSIMD stripes."""
    n = len(idx)
    a = np.zeros((16, width), np.int16)
    a[np.arange(n) % 16, np.arange(n) // 16] = idx
    return np.tile(a, (8, 1))


def prepare_in_maps(inputs, tags, emb_table, W1, b1, W2, b2,
                    start_trans, end_trans, transitions, pad_value=0):
    inputs = np.asarray(inputs)
    tags = np.asarray(tags)
    # fast path requires every token real (any word-feature id != 0)
    assert bool(((inputs != 0).sum(-1) != 0).all()), \
        "kernel fast path assumes all-ones mask"

    tableb = np.ascontiguousarray(
        np.asarray(emb_table, np.float32).astype(ml_dtypes.bfloat16))
    params = np.zeros((128, 5), np.float32)
    params[0:T, 0] = np.asarray(start_trans, np.float32)
    params[0:T, 1] = np.asarray(end_trans, np.float32)
    params[0:H, 2] = np.asarray(b1, np.float32)
    params[0:T, 3] = np.asarray(b2, np.float32) - np.float32(np.log(T))
    params[0:T, 4] = np.asarray(b2, np.float32)
    w1bf = np.ascontiguousarray(
        np.asarray(W1, np.float32).astype(ml_dtypes.bfloat16))
    w2bf = np.ascontiguousarray(
        np.asarray(W2, np.float32).astype(ml_dtypes.bfloat16))
    trans = np.ascontiguousarray(np.asarray(transitions, np.float32))
    transT = np.ascontiguousarray(trans.T)
    tflat = np.tile(np.append(trans.ravel(), np.float32(0.0)), (128, 1))
    tflat = np.ascontiguousarray(tflat, np.float32)
    bdg = np.zeros((128, 8), np.float32)
    bdg[np.arange(8) * 16, np.arange(8)] = 1.0
    rows = np.concatenate([np.asarray(start_trans, np.float32),
                           np.asarray(end_trans, np.float32)]).reshape(1, -1)

    in_maps = []
    for c in range(NCORES):
        ids_c = inputs[c * BC:(c + 1) * BC]          # [BC, S, W]
        tags_c = np.asarray(tags[c * BC:(c + 1) * BC], np.int64)  # [BC, S]
        ids_t = np.asarray(ids_c.transpose(1, 0, 2).reshape(N, W), np.int64)
        idx = np.zeros((128, NGG * IXW), np.int16)
        for g in range(NGG):
            ids_g = ids_t[g * GTOK:(g + 1) * GTOK]   # [GTOK, W]
            sid = ids_g.T.reshape(LK)                # slot i = w*GTOK + t
            chunk = sid >> 15
            local = sid & (CHUNK - 1)
            perm = np.empty(LK, np.int64)
            base = g * IXW
            for cc in range(4):
                pos = np.flatnonzero(chunk == cc)
                cnt = len(pos)
                assert cnt <= TC[cc], f"chunk {cc} count {cnt} > {TC[cc]}"
                stream = np.full(TC[cc], pad_value, np.int16)
                stream[:cnt] = local[pos]
                xoff = base + sum(TC[c2_] // 16 for c2_ in range(cc))
                idx[:, xoff:xoff + TC[cc] // 16] = _wrap16(stream,
                                                           TC[cc] // 16)
                perm[pos] = TSTART[cc] + np.arange(cnt)
            s2off = base + sum(c2_ // 16 for c2_ in TC)
            idx[:, s2off:s2off + LK // 16] = _wrap16(perm, LK // 16)
        tags_tm = tags_c.T                            # [S, BC]
        tags_flat = tags_tm.reshape(N)
        ohm = np.zeros((T, N), np.float32)
        ohm[tags_flat, np.arange(N)] = 1.0
        ohmb = np.ascontiguousarray(ohm.astype(ml_dtypes.bfloat16))
        # pair indices, padded with a dummy at s = S-1
        pair = np.full((BC, S), PAIR_PAD, np.int64)
        pair[:, :S - 1] = tags_c[:, :-1] * T + tags_c[:, 1:]
        pw = pair.reshape(8, 8, S // 16, 16).transpose(0, 3, 1, 2)
        pw = np.ascontiguousarray(pw.reshape(128, NPAIR // 16), np.int16)
        in_maps.append({
            "idx": idx, "ohmb": ohmb, "pairs": pw,
            "tableb": tableb, "params": params, "w1b": w1bf, "w2b": w2bf,
            "trans": trans, "transT": transT, "tflat": tflat,
            "bd": bdg, "rows": rows,
        })
    return in_maps


_CACHE = {}


def kernel(**inputs):
    from concourse.bass_utils import run_bass_kernel_spmd
    if "nc" not in _CACHE:
        _CACHE["nc"] = build_program()
    nc = _CACHE["nc"]
    in_maps = prepare_in_maps(**inputs)
    res = run_bass_kernel_spmd(nc, in_maps, list(range(NCORES)))
    out = np.concatenate([res.results[c]["out"].reshape(BC)
                          for c in range(NCORES)])
    return out.astype(np.float32)


# revision 37
# speedup vs baseline: 1.1333x; 1.1333x over previous
"""CRF tagger loss kernel for Trainium2 (8 NeuronCores, data-parallel over batch).

Self-contained: hardcodes all shapes. kernel(**inputs) takes full inputs,
shards batch over 8 cores, runs one SPMD Bass program, returns [B] f32 loss.

Embedding gather: two-stage dma_gather with descriptor generation spread
across all 4 SWDGE queues (each queue runs on its own Q7 core pair, so the
4 queues generate descriptors concurrently). Stage 1 gathers bf16 table
rows chunk-compactly (int16 row index per 32768-row chunk, zero-padded to
static counts); stage 2 un-permutes via SBUF-source transpose gathers
(split over the queues so that per-queue descriptor counts balance:
2304+1024 = 256+3072 = 3328 per queue per group).

Forward algorithm: split at the sequence midpoint into a forward chain
(alpha, 256 steps) and a backward chain (beta, 256 steps) that run
concurrently, halving the serial latency. Both run in the scaled domain
(bf16 state, f32 PSUM) with a log accumulator and periodic rescale:
  fwd: A_t = (expT^T A_{t-1}) * E_t
  bwd: Z_{t-1} = expT (E_t * Z_t)
  denom = ln(sum_i A_mid[i] * Z_mid[i]) + Lf + Lb + S*lnT
"""
import os
import sys

sys.path.insert(0, "/opt/trn_rl_repo")

import numpy as np
import ml_dtypes

import concourse.bacc as bacc
import concourse.bass as bass
import concourse.tile as tile
from concourse import mybir
from concourse.bass import AP

# ---- problem dims (hardcoded from the nn_CRFTagger problem) ----
B, S, W, V, E, H, T = 512, 512, 3, 100000, 128, 100, 64
NCORES = 8
BC = B // NCORES          # sequences per core = 64
N = BC * S                # tokens per core = 32768 (time-major: t = s*BC + b)
GTOK = 2048               # tokens per gather group
NGG = N // GTOK           # gather groups = 16
LK = 3 * GTOK             # lookups per group = 6144
CHUNK = 32768             # table rows addressable per int16 gather
TC = [2176, 2176, 2176, 256]   # static per-chunk gather capacity
TSTART = [0, 2176, 4352, 6528]
NSTAGE = 6784             # staged slots per group (= sum(TC), 53*128)
S2SPLIT = [1024, 1024, 1024, 3072]   # stage-2 idx per queue (128-aligned)
IXW = sum(c // 16 for c in TC) + 3 * 2048 // 16   # idx cols per group = 808
WIN = 512                 # tokens per window (= 8 time steps x 64 b)
NW = N // WIN             # windows = 64
NWH = NW // 2             # windows per scan direction = 32
WPG = GTOK // WIN         # windows per group = 4
SPW = WIN // BC           # time steps per window = 8
LA = 1                    # window lookahead (emission ahead of scan)
EMBUFS = 8                # rotating emission-window buffers
RESCALE = 32              # scan rescale cadence
F32 = mybir.dt.float32
BF16 = mybir.dt.bfloat16
PAIR_PAD = T * T          # dummy pair index -> gathers 0.0
NPAIR = (S * BC) // NCORES   # pairs per 16-partition stripe = 4096
PCHUNK = 4                # pair-gather chunks
NQ = 4                    # SWDGE queues


def build_program():
    nc = bacc.Bacc("TRN2", target_bir_lowering=False, debug=False,
                   num_swdge_queues=NQ)

    # ---- DRAM I/O ----
    idx_d = nc.dram_tensor("idx", [128, NGG * IXW], mybir.dt.int16,
                           kind="ExternalInput")
    ohm_d = nc.dram_tensor("ohmb", [T, N], BF16, kind="ExternalInput")
    pairs_d = nc.dram_tensor("pairs", [128, NPAIR // 16],
                             mybir.dt.int16, kind="ExternalInput")
    table_d = nc.dram_tensor("tableb", [V, E], BF16, kind="ExternalInput")
    params_d = nc.dram_tensor("params", [128, 5], F32, kind="ExternalInput")
    w1b_d = nc.dram_tensor("w1b", [E, H], BF16, kind="ExternalInput")
    w2b_d = nc.dram_tensor("w2b", [H, T], BF16, kind="ExternalInput")
    trans_d = nc.dram_tensor("trans", [T, T], F32, kind="ExternalInput")
    transT_d = nc.dram_tensor("transT", [T, T], F32, kind="ExternalInput")
    tflat_d = nc.dram_tensor("tflat", [128, T * T + 1], F32,
                             kind="ExternalInput")
    bd_d = nc.dram_tensor("bd", [128, 8], F32, kind="ExternalInput")
    rows_d = nc.dram_tensor("rows", [1, 2 * T], F32, kind="ExternalInput")
    out_d = nc.dram_tensor("out", [1, BC], F32, kind="ExternalOutput")

    with tile.TileContext(nc) as tc:
        with (
            tc.tile_pool(name="const", bufs=1) as cp,
            tc.tile_pool(name="emtiles", bufs=6) as emp,
            tc.tile_pool(name="stage", bufs=6) as stp,
            tc.tile_pool(name="ixp", bufs=6) as ixp,
            tc.tile_pool(name="g2p", bufs=4) as g2p,
            tc.tile_pool(name="hpool", bufs=3) as hp,
            tc.tile_pool(name="ohm", bufs=5) as op_,
            tc.tile_pool(name="upool", bufs=8) as up,
            tc.tile_pool(name="small", bufs=4) as sp,
            tc.tile_pool(name="pgp", bufs=4) as pgp,
            tc.tile_pool(name="psW", bufs=4, space="PSUM") as psW,
            tc.tile_pool(name="psS", bufs=4, space="PSUM") as psS,
        ):
            # ---- constants to SBUF ----
            params = cp.tile([128, 5], F32)
            nc.sync.dma_start(out=params[:], in_=params_d[:])
            w1b = cp.tile([E, H], BF16)
            nc.sync.dma_start(out=w1b[:], in_=w1b_d[:])
            w2b = cp.tile([H, T], BF16)
            nc.sync.dma_start(out=w2b[:], in_=w2b_d[:])
            trans = cp.tile([T, T], F32)
            nc.sync.dma_start(out=trans[:], in_=trans_d[:])
            transT = cp.tile([T, T], F32)
            nc.sync.dma_start(out=transT[:], in_=transT_d[:])
            bd = cp.tile([128, 8], F32)
            nc.sync.dma_start(out=bd[:], in_=bd_d[:])
            pairs = cp.tile([128, NPAIR // 16], mybir.dt.int16)
            nc.sync.dma_start(out=pairs[:], in_=pairs_d[:])
            tflat = cp.tile([128, T * T + 1], F32)
            rows = cp.tile([1, 2 * T], F32)
            nc.sync.dma_start(out=rows[:], in_=rows_d[:])

            expT = cp.tile([T, T], BF16)
            nc.scalar.activation(out=expT[:], in_=trans[:],
                                 func=mybir.ActivationFunctionType.Exp)
            expTT = cp.tile([T, T], BF16)
            nc.scalar.activation(out=expTT[:], in_=transT[:],
                                 func=mybir.ActivationFunctionType.Exp)
            # exp(start/end) broadcast to [T, BC] via outer-product matmul
            # (tensor_scalar with a per-partition scalar vector is ~50us on
            # DVE hardware -- avoid it on the critical path).
            expRows = cp.tile([1, 2 * T], F32)
            nc.scalar.activation(out=expRows[:], in_=rows[:],
                                 func=mybir.ActivationFunctionType.Exp)
            onesb_row = cp.tile([1, BC], F32)
            nc.vector.memset(onesb_row[:], 1.0)
            expStartB = cp.tile([T, BC], BF16)
            expEndB = cp.tile([T, BC], BF16)
            for _i, _dst in ((0, expStartB), (1, expEndB)):
                _ps = psS.tile([T, BC], F32, tag="scan")
                nc.tensor.matmul(_ps[:], lhsT=expRows[0:1, _i * T:
                                                       (_i + 1) * T],
                                 rhs=onesb_row[:])
                nc.vector.tensor_copy(out=_dst[:], in_=_ps[:])
            stb = cp.tile([T, 1], BF16)          # start_trans bf16
            nc.vector.tensor_copy(out=stb[:], in_=params[0:T, 0:1])
            etb = cp.tile([T, 1], BF16)          # end_trans bf16
            nc.vector.tensor_copy(out=etb[:], in_=params[0:T, 1:2])

            ones_rb = cp.tile([1, T], BF16)
            nc.vector.memset(ones_rb[:], 1.0)
            ones_rf = cp.tile([1, T], F32)
            nc.vector.memset(ones_rf[:], 1.0)
            ones_cb = cp.tile([T, 1], BF16)
            nc.vector.memset(ones_cb[:], 1.0)
            ones_cf = cp.tile([T, 1], F32)
            nc.vector.memset(ones_cf[:], 1.0)

            acc2 = cp.tile([T, BC], F32)       # sum_s ohm*(em+b2) accumulator
            nc.vector.memset(acc2[:], 0.0)
            # rescale row-0 values, logged in one batched Ln at the end
            # (fwd events in cols [0:8*BC), bwd in [8*BC:16*BC))
            lnbuf = cp.tile([1, 16 * BC], BF16)
            nc.vector.memset(lnbuf[:], 1.0)
            se0 = cp.tile([1, BC], F32)        # start-term
            se1 = cp.tile([1, BC], F32)        # end-term
            nc.vector.memset(se0[:], 0.0)
            nc.vector.memset(se1[:], 0.0)
            tsum = cp.tile([1, BC], F32)
            nc.vector.memset(tsum[:], 0.0)

            st_tiles = {}
            ix_tiles = {}
            g2_tiles = {}
            em_w = {}
            fwd = {"A": None}
            bwd = {"y": None}

            def stage1(g):
                ix = ixp.tile([128, IXW], mybir.dt.int16, tag="ix")
                nc.sync.dma_start(out=ix[:], in_=idx_d[:, g * IXW:
                                                       (g + 1) * IXW])
                ix_tiles[g] = ix
                st = stp.tile([128, NSTAGE], BF16, tag="st")
                stv = st[:].rearrange("p (r e) -> p r e", e=128)
                # queue order must match stage2's (0,1,2,3): the 8 DMASW
                # sems rotate round-robin and each locks to one SWDGE queue.
                for c in (0, 1, 2, 3):
                    xoff = sum(TC[cc] // 16 for cc in range(c))
                    rows = min(CHUNK, V - c * CHUNK)
                    nc.gpsimd.dma_gather(
                        out_ap=stv[:, TSTART[c] // 128:
                                   (TSTART[c] + TC[c]) // 128, :],
                        in_ap=table_d[c * CHUNK:c * CHUNK + rows, :],
                        idxs_ap=ix[:, xoff:xoff + TC[c] // 16],
                        num_idxs=TC[c], num_idxs_reg=TC[c], elem_size=E,
                        single_packet=False, queue_num=c % NQ)
                st_tiles[g] = st

            def stage2(g):
                st = st_tiles.pop(g)
                ix = ix_tiles.pop(g)
                s2base = sum(c // 16 for c in TC)
                g2 = g2p.tile([128, 1, LK], BF16, tag="g2")
                off = 0
                for q in range(NQ):
                    n = S2SPLIT[q]
                    xoff = s2base + off // 16
                    nc.gpsimd.dma_gather(
                        out_ap=g2[:, :, off:off + n],
                        in_ap=st[:], idxs_ap=ix[:, xoff:xoff + n // 16],
                        num_idxs=n, num_idxs_reg=n, elem_size=E,
                        transpose=True, single_packet=False, queue_num=q,
                        sbuf_tokens_per_rank=128, sbuf_free_dim_per_rank=256,
                        sbuf_free_dim_pad_per_rank=0, sbuf_byte_offset=0)
                    off += n
                g2_tiles[g] = g2

            def emission(w):
                g2 = g2_tiles[w // WPG]
                col = (w % WPG) * WIN
                h_ps = psW.tile([H, WIN], F32, tag="psw")
                for i in range(W):
                    nc.tensor.matmul(h_ps[:], lhsT=w1b[:],
                                     rhs=g2[:, 0, i * GTOK + col:
                                            i * GTOK + col + WIN],
                                     start=(i == 0), stop=(i == W - 1))
                h_sb = hp.tile([H, WIN], BF16, tag="h")
                nc.scalar.activation(out=h_sb[:], in_=h_ps[:],
                                     func=mybir.ActivationFunctionType.Tanh,
                                     bias=params[0:H, 2:3])
                em_ps = psW.tile([T, WIN], F32, tag="psw")
                nc.tensor.matmul(em_ps[:], lhsT=w2b[:], rhs=h_sb[:])
                # exp(em + b2 - logT) -> emission-exp window (rotating buffer)
                emt = emp.tile([T, WIN], BF16, tag="em")
                em_w[w] = emt
                nc.scalar.activation(out=emt[:], in_=em_ps[:],
                                     func=mybir.ActivationFunctionType.Exp,
                                     bias=params[0:T, 3:4])
                # numerator: gold-path emission scores via host one-hot tags
                ohm = op_.tile([T, WIN], BF16, tag="ohm")
                nc.sync.dma_start(out=ohm[:],
                                  in_=ohm_d[:, w * WIN:(w + 1) * WIN])
                tmp = op_.tile([T, WIN], BF16, tag="tmp")
                nc.vector.scalar_tensor_tensor(
                    out=tmp[:], in0=em_ps[:], scalar=params[0:T, 4:5],
                    in1=ohm[:], op0=mybir.AluOpType.add,
                    op1=mybir.AluOpType.mult)
                red = op_.tile([T, BC], F32, tag="red")
                nc.vector.tensor_reduce(
                    out=red[:],
                    in_=tmp[:].rearrange("p (s b) -> p b s", s=SPW),
                    axis=mybir.AxisListType.X, op=mybir.AluOpType.add)
                nc.vector.tensor_tensor(out=acc2[:], in0=acc2[:], in1=red[:],
                                        op=mybir.AluOpType.add)
                if w == 0:
                    s0_ps = psS.tile([1, BC], F32, tag="scan")
                    nc.tensor.matmul(s0_ps[:], lhsT=stb[:],
                                     rhs=ohm[:, 0:BC])
                    nc.vector.tensor_copy(out=se0[:], in_=s0_ps[:])
                if w == NW - 1:
                    s1_ps = psS.tile([1, BC], F32, tag="scan")
                    nc.tensor.matmul(s1_ps[:], lhsT=etb[:],
                                     rhs=ohm[:, WIN - BC:WIN])
                    nc.vector.tensor_copy(out=se1[:], in_=s1_ps[:])

            def rescale(state, slot):
                """Divide state (SBUF [T,BC] bf16) by its row 0; stash the
                row-0 values in lnbuf slot for one batched Ln at the end."""
                rec = sp.tile([1, BC], F32, tag="rec")
                nc.vector.reciprocal(out=rec[:], in_=state[0:1, :])
                nc.vector.tensor_copy(
                    out=lnbuf[0:1, slot * BC:(slot + 1) * BC],
                    in_=state[0:1, :])
                rb_ps = psS.tile([T, BC], F32, tag="scan")
                nc.tensor.matmul(rb_ps[:], lhsT=ones_rf[:], rhs=rec[:])
                s2 = up.tile([T, BC], BF16, tag="U")
                nc.vector.tensor_tensor(out=s2[:], in0=rb_ps[:], in1=state[:],
                                        op=mybir.AluOpType.mult)
                return s2

            def scan_fwd_window(w):
                emw = em_w.pop(w)
                for sl in range(SPW):
                    s = w * SPW + sl
                    col = sl * BC
                    if s == 0:
                        A = up.tile([T, BC], BF16, tag="U")
                        nc.vector.tensor_tensor(
                            out=A[:], in0=emw[:, col:col + BC],
                            in1=expStartB[:], op=mybir.AluOpType.mult)
                        fwd["A"] = A
                        continue
                    y_ps = psS.tile([T, BC], F32, tag="scan")
                    nc.tensor.matmul(y_ps[:], lhsT=expT[:], rhs=fwd["A"][:])
                    A = up.tile([T, BC], BF16, tag="U")
                    nc.vector.tensor_tensor(out=A[:], in0=y_ps[:],
                                            in1=emw[:, col:col + BC],
                                            op=mybir.AluOpType.mult)
                    fwd["A"] = A
                    if s % RESCALE == RESCALE - 1:
                        fwd["A"] = rescale(fwd["A"], s // RESCALE)

            def scan_bwd_window(w):
                emw = em_w.pop(w)
                for sl in reversed(range(SPW)):
                    s = w * SPW + sl
                    k = (S - 1) - s
                    col = sl * BC
                    Wt = up.tile([T, BC], BF16, tag="U")
                    if s == S - 1:
                        nc.vector.tensor_tensor(
                            out=Wt[:], in0=emw[:, col:col + BC],
                            in1=expEndB[:], op=mybir.AluOpType.mult)
                    else:
                        nc.vector.tensor_tensor(out=Wt[:], in0=bwd["y"][:],
                                                in1=emw[:, col:col + BC],
                                                op=mybir.AluOpType.mult)
                    if k % RESCALE == RESCALE - 1:
                        Wt = rescale(Wt, 8 + k // RESCALE)
                    y_ps = psS.tile([T, BC], F32, tag="scan")
                    nc.tensor.matmul(y_ps[:], lhsT=expTT[:], rhs=Wt[:])
                    bwd["y"] = y_ps

            def pairs_block():
                # numerator: transition-pair scores via ap_gather. Issued
                # right after the last dma_gather so the ap_gather ext-isa
                # IRAM swap lands in the gather tail, off the critical path.
                pred = cp.tile([128, 8], F32)
                cn = NPAIR // PCHUNK              # 512 idxs per chunk
                for c in range(PCHUNK):
                    pg = pgp.tile([128, cn], F32, tag="pg")
                    nc.gpsimd.ap_gather(
                        out_ap=pg[:].rearrange("p (n o) -> p n o", o=1),
                        in_ap=tflat[:].rearrange("p (n o) -> p n o", o=1),
                        idxs_ap=pairs[:, c * (cn // 16):(c + 1) * (cn // 16)],
                        channels=128, num_elems=T * T + 1, d=1, num_idxs=cn,
                    )
                    nc.vector.tensor_reduce(
                        out=pred[:, 2 * c:2 * c + 2],
                        in_=pg[:].rearrange("p (g s) -> p g s", g=2),
                        axis=mybir.AxisListType.X, op=mybir.AluOpType.add)
                ts_ps = psS.tile([8, 8], F32, tag="scan")
                nc.tensor.matmul(ts_ps[:], lhsT=bd[:], rhs=pred[:])
                ts8 = sp.tile([8, 8], F32)
                nc.vector.tensor_copy(out=ts8[:], in_=ts_ps[:])
                nc.sync.dma_start(
                    out=tsum[:].rearrange("p (g b) -> p g b", g=8), in_=ts8[:])

            # ---- main pipeline: emissions + fwd/bwd scans ----
            # stage1 runs two quads ahead of its stage2, so stage2's wait on
            # stage1 DMA completion is satisfied long before dispatch.
            stage1(0)
            stage1(NGG - 1)
            stage1(1)
            stage1(NGG - 2)
            for p in range(NWH + LA):
                if p < NWH and p % WPG == 0:
                    qd = p // WPG
                    stage2(qd)
                    stage2(NGG - 1 - qd)
                    if qd + 2 < NGG // 2:
                        stage1(qd + 2)
                        stage1(NGG - 3 - qd)
                    if qd == NGG // 2 - 2:
                        nc.sync.dma_start(out=tflat[:], in_=tflat_d[:])
                    if qd == NGG // 2 - 1:
                        pairs_block()
                if p < NWH:
                    emission(p)
                    emission(NW - 1 - p)
                if p >= LA:
                    scan_fwd_window(p - LA)
                    scan_bwd_window(NW - 1 - (p - LA))

            # ---- finals ----
            # denominator: ln(sum_i A_mid * Z_mid) + Lf + Lb + S*lnT
            P = sp.tile([T, BC], BF16)
            nc.vector.tensor_tensor(out=P[:], in0=bwd["y"][:],
                                    in1=fwd["A"][:], op=mybir.AluOpType.mult)
            dn_ps = psS.tile([1, BC], F32, tag="scan")
            nc.tensor.matmul(dn_ps[:], lhsT=ones_cb[:], rhs=P[:])
            dlog = sp.tile([1, BC], F32, tag="dlog")
            nc.scalar.activation(out=dlog[:], in_=dn_ps[:],
                                 func=mybir.ActivationFunctionType.Ln)
            lnv = cp.tile([1, 16 * BC], BF16)
            nc.scalar.activation(out=lnv[:], in_=lnbuf[:],
                                 func=mybir.ActivationFunctionType.Ln)
            Lsum = sp.tile([1, BC], F32, tag="denom")
            nc.vector.tensor_reduce(
                out=Lsum[:],
                in_=lnv[:].rearrange("p (k b) -> p b k", k=16),
                axis=mybir.AxisListType.X, op=mybir.AluOpType.add)
            denom = sp.tile([1, BC], F32, tag="denom")
            nc.vector.tensor_tensor(out=denom[:], in0=dlog[:], in1=Lsum[:],
                                    op=mybir.AluOpType.add)
            nc.vector.tensor_scalar_add(out=denom[:], in0=denom[:],
                                        scalar1=float(S * np.log(T)))

            esc_ps = psS.tile([1, BC], F32, tag="scan")
            nc.tensor.matmul(esc_ps[:], lhsT=ones_cf[:], rhs=acc2[:])
            num = sp.tile([1, BC], F32, tag="num")
            nc.vector.tensor_tensor(out=num[:], in0=esc_ps[:], in1=tsum[:],
                                    op=mybir.AluOpType.add)
            nc.vector.tensor_tensor(out=num[:], in0=num[:], in1=se0[:],
                                    op=mybir.AluOpType.add)
            nc.vector.tensor_tensor(out=num[:], in0=num[:], in1=se1[:],
                                    op=mybir.AluOpType.add)
            outv = sp.tile([1, BC], F32, tag="outv")
            nc.vector.tensor_tensor(out=outv[:], in0=denom[:], in1=num[:],
                                    op=mybir.AluOpType.subtract)
            nc.sync.dma_start(out=out_d[:], in_=outv[:])

    nc.compile()
    return nc


def _wrap16(idx, width):
    """idx list -> [128, width] int16: i -> (partition i%16, free i//16),
    replicated across the 8 GPSIMD stripes."""
    n = len(idx)
    a = np.zeros((16, width), np.int16)
    a[np.arange(n) % 16, np.arange(n) // 16] = idx
    return np.tile(a, (8, 1))


def prepare_in_maps(inputs, tags, emb_table, W1, b1, W2, b2,
                    start_trans, end_trans, transitions, pad_value=0):
    inputs = np.asarray(inputs)
    tags = np.asarray(tags)
    # fast path requires every token real (any word-feature id != 0)
    assert bool(((inputs != 0).sum(-1) != 0).all()), \
        "kernel fast path assumes all-ones mask"

    tableb = np.ascontiguousarray(
        np.asarray(emb_table, np.float32).astype(ml_dtypes.bfloat16))
    params = np.zeros((128, 5), np.float32)
    params[0:T, 0] = np.asarray(start_trans, np.float32)
    params[0:T, 1] = np.asarray(end_trans, np.float32)
    params[0:H, 2] = np.asarray(b1, np.float32)
    params[0:T, 3] = np.asarray(b2, np.float32) - np.float32(np.log(T))
    params[0:T, 4] = np.asarray(b2, np.float32)
    w1bf = np.ascontiguousarray(
        np.asarray(W1, np.float32).astype(ml_dtypes.bfloat16))
    w2bf = np.ascontiguousarray(
        np.asarray(W2, np.float32).astype(ml_dtypes.bfloat16))
    trans = np.ascontiguousarray(np.asarray(transitions, np.float32))
    transT = np.ascontiguousarray(trans.T)
    tflat = np.tile(np.append(trans.ravel(), np.float32(0.0)), (128, 1))
    tflat = np.ascontiguousarray(tflat, np.float32)
    bdg = np.zeros((128, 8), np.float32)
    bdg[np.arange(8) * 16, np.arange(8)] = 1.0
    rows = np.concatenate([np.asarray(start_trans, np.float32),
                           np.asarray(end_trans, np.float32)]).reshape(1, -1)

    in_maps = []
    for c in range(NCORES):
        ids_c = inputs[c * BC:(c + 1) * BC]          # [BC, S, W]
        tags_c = np.asarray(tags[c * BC:(c + 1) * BC], np.int64)  # [BC, S]
        ids_t = np.asarray(ids_c.transpose(1, 0, 2).reshape(N, W), np.int64)
        idx = np.zeros((128, NGG * IXW), np.int16)
        for g in range(NGG):
            ids_g = ids_t[g * GTOK:(g + 1) * GTOK]   # [GTOK, W]
            sid = ids_g.T.reshape(LK)                # slot i = w*GTOK + t
            chunk = sid >> 15
            local = sid & (CHUNK - 1)
            perm = np.empty(LK, np.int64)
            base = g * IXW
            for cc in range(4):
                pos = np.flatnonzero(chunk == cc)
                cnt = len(pos)
                assert cnt <= TC[cc], f"chunk {cc} count {cnt} > {TC[cc]}"
                stream = np.full(TC[cc], pad_value, np.int16)
                stream[:cnt] = local[pos]
                xoff = base + sum(TC[c2_] // 16 for c2_ in range(cc))
                idx[:, xoff:xoff + TC[cc] // 16] = _wrap16(stream,
                                                           TC[cc] // 16)
                perm[pos] = TSTART[cc] + np.arange(cnt)
            s2off = base + sum(c2_ // 16 for c2_ in TC)
            idx[:, s2off:s2off + LK // 16] = _wrap16(perm, LK // 16)
        tags_tm = tags_c.T                            # [S, BC]
        tags_flat = tags_tm.reshape(N)
        ohm = np.zeros((T, N), np.float32)
        ohm[tags_flat, np.arange(N)] = 1.0
        ohmb = np.ascontiguousarray(ohm.astype(ml_dtypes.bfloat16))
        # pair indices, padded with a dummy at s = S-1
        pair = np.full((BC, S), PAIR_PAD, np.int64)
        pair[:, :S - 1] = tags_c[:, :-1] * T + tags_c[:, 1:]
        pw = pair.reshape(8, 8, S // 16, 16).transpose(0, 3, 1, 2)
        pw = np.ascontiguousarray(pw.reshape(128, NPAIR // 16), np.int16)
        in_maps.append({
            "idx": idx, "ohmb": ohmb, "pairs": pw,
            "tableb": tableb, "params": params, "w1b": w1bf, "w2b": w2bf,
            "trans": trans, "transT": transT, "tflat": tflat,
            "bd": bdg, "rows": rows,
        })
    return in_maps


_CACHE = {}


def kernel(**inputs):
    from concourse.bass_utils import run_bass_kernel_spmd
    if "nc" not in _CACHE:
        _CACHE["nc"] = build_program()
    nc = _CACHE["nc"]
    in_maps = prepare_in_maps(**inputs)
    res = run_bass_kernel_spmd(nc, in_maps, list(range(NCORES)))
    out = np.concatenate([res.results[c]["out"].reshape(BC)
                          for c in range(NCORES)])
    return out.astype(np.float32)
